# revision 2
# baseline (speedup 1.0000x reference)
"""Trainium2 Bass kernel: segmented mean-pool over ragged bags.

Problem (nn_Aggregator): samples [131072, 512] f32, bags_num_samples [64] i64.
Bag i owns a contiguous run of rows; output[i] = mean of its rows.

Strategy (bag-parallel data parallelism across 8 NeuronCores):
- Shard rows evenly: 16384 rows per core.
- Per core, for each 128-row tile: build a one-hot indicator [128, 64]
  on-chip (DVE is_equal of per-row segment id against a bag-index iota) and
  accumulate indicator.T @ tile into a PSUM [64, 512] partial via TensorE.
- Per-core partials [64, 512] are summed on the host (8 tiny arrays) and
  divided by the bag counts, matching the reference's fp32 semantics.

All data-dependent behavior (ragged boundaries) lives in per-core *data*
(segment ids), so a single SPMD program serves all cores.
"""
import numpy as np

from concourse import mybir
from concourse.bacc import Bacc
from concourse.tile import TileContext
from concourse.bass_utils import run_bass_kernel_spmd

# Hardcoded problem shape (harness contract).
B = 64          # bags
D = 512         # feature dim
T = 131072      # total rows
N_CORES = 8
P = 128         # SBUF partitions
T_LOCAL = T // N_CORES      # 16384 rows per core
NTILES = T_LOCAL // P       # 128 tiles of 128 rows
CHUNK = 8                   # tiles per DMA (8 * 256 KiB = 2 MiB)
NCHUNKS = NTILES // CHUNK   # 16

_NC_CACHE = None


def build_program(loop_repeats: int = 1):
    """loop_repeats > 1 wraps the body in a hardware loop that redoes the
    identical work (same inputs/outputs) — used only for slope-based timing."""
    nc = Bacc()
    x = nc.dram_tensor("x", [T_LOCAL, D], mybir.dt.float32, kind="ExternalInput")
    # consts[:, :NTILES] = per-tile segment ids (transposed), consts[:, NTILES:]
    # = bag-index iota. One tensor so downstream ops depend on a single DMA.
    consts = nc.dram_tensor(
        "consts", [P, NTILES + B], mybir.dt.float32, kind="ExternalInput"
    )
    out = nc.dram_tensor("out", [B, D], mybir.dt.float32, kind="ExternalOutput")

    # [128, NTILES, 512]: element [p, c, d] = x[c*128 + p, d]
    x_view = x.rearrange("(c p) d -> p c d", p=P)

    with TileContext(nc) as tc:
        with (
            tc.tile_pool(name="const", bufs=1) as const_pool,
            tc.tile_pool(name="xin", bufs=3) as x_pool,
            tc.tile_pool(name="ind", bufs=3) as ind_pool,
            tc.tile_pool(name="psum", bufs=1, space="PSUM") as psum_pool,
            tc.tile_pool(name="outp", bufs=1) as out_pool,
        ):
            const_tile = const_pool.tile([P, NTILES + B], mybir.dt.float32)
            nc.sync.dma_start(out=const_tile[:], in_=consts[:])
            seg_tile = const_tile[:, :NTILES]
            iota_tile = const_tile[:, NTILES:]

            def body():
                acc = psum_pool.tile([B, D], mybir.dt.float32)
                for ch in range(NCHUNKS):
                    xt = x_pool.tile([P, CHUNK * D], mybir.dt.float32)
                    nc.sync.dma_start(
                        out=xt[:].rearrange("p (c d) -> p c d", c=CHUNK),
                        in_=x_view[:, ch * CHUNK : (ch + 1) * CHUNK, :],
                    )
                    ind = ind_pool.tile([P, CHUNK * B], mybir.dt.float32)
                    for c in range(CHUNK):
                        k = ch * CHUNK + c
                        nc.vector.tensor_tensor(
                            out=ind[:, c * B : (c + 1) * B],
                            in0=seg_tile[:, k : k + 1].to_broadcast([P, B]),
                            in1=iota_tile[:, :],
                            op=mybir.AluOpType.is_equal,
                        )
                    for c in range(CHUNK):
                        k = ch * CHUNK + c
                        nc.tensor.matmul(
                            acc[:],
                            lhsT=ind[:, c * B : (c + 1) * B],
                            rhs=xt[:, c * D : (c + 1) * D],
                            start=(k == 0),
                            stop=(k == NTILES - 1),
                        )

                out_sb = out_pool.tile([B, D], mybir.dt.float32)
                nc.vector.tensor_copy(out=out_sb[:], in_=acc[:])
                nc.sync.dma_start(out=out[:], in_=out_sb[:])

            if loop_repeats == 1:
                body()
            else:
                with tc.For_i(0, loop_repeats, 1):
                    body()

    nc.finalize()
    return nc


def _segment_ids(bags_num_samples: np.ndarray) -> np.ndarray:
    """Replicates jnp.repeat(arange(B), counts, total_repeat_length=T):
    truncate if the full repeat exceeds T, pad with the last value if short."""
    counts = np.asarray(bags_num_samples, dtype=np.int64)
    reps = np.repeat(np.arange(counts.shape[0], dtype=np.int64), np.maximum(counts, 0))
    if reps.size >= T:
        return reps[:T]
    pad_val = reps[-1] if reps.size else np.int64(0)
    return np.concatenate([reps, np.full(T - reps.size, pad_val, dtype=np.int64)])


def prepare_in_maps(samples: np.ndarray, bags_num_samples: np.ndarray):
    samples = np.ascontiguousarray(np.asarray(samples, dtype=np.float32))
    seg = _segment_ids(bags_num_samples).astype(np.float32)
    biota = np.broadcast_to(np.arange(B, dtype=np.float32)[None, :], (P, B))
    in_maps = []
    for i in range(N_CORES):
        lo = i * T_LOCAL
        seg_t = seg[lo : lo + T_LOCAL].reshape(NTILES, P).T  # [128, NTILES]
        consts = np.ascontiguousarray(np.concatenate([seg_t, biota], axis=1))
        in_maps.append({"x": samples[lo : lo + T_LOCAL], "consts": consts})
    return in_maps


def _combine(partials, bags_num_samples: np.ndarray) -> np.ndarray:
    sums = np.sum(np.stack(partials), axis=0, dtype=np.float32)
    counts_f = np.asarray(bags_num_samples)[:, None].astype(np.float32)
    with np.errstate(divide="ignore", invalid="ignore"):
        return (sums / counts_f).astype(np.float32)


def kernel(samples: np.ndarray, bags_num_samples: np.ndarray) -> np.ndarray:
    global _NC_CACHE
    if _NC_CACHE is None:
        _NC_CACHE = build_program()
    in_maps = prepare_in_maps(samples, bags_num_samples)
    res = run_bass_kernel_spmd(_NC_CACHE, in_maps, list(range(N_CORES)))
    partials = [res.results[i]["out"] for i in range(N_CORES)]
    return _combine(partials, bags_num_samples)


# revision 4
# speedup vs baseline: 1.5389x; 1.5389x over previous
"""Trainium2 Bass kernel: segmented mean-pool over ragged bags.

Problem (nn_Aggregator): samples [131072, 512] f32, bags_num_samples [64] i64.
Bag i owns a contiguous run of rows; output[i] = mean of its rows.

Strategy (bag-parallel data parallelism across 8 NeuronCores):
- Shard rows evenly: 16384 rows per core.
- Per core, for each 128-row tile: build a one-hot indicator [128, 64]
  on-chip (DVE is_equal of per-row segment id against a bag-index iota) and
  accumulate indicator.T @ tile into a PSUM [64, 512] partial via TensorE.
- Per-core partials [64, 512] are summed on the host (8 tiny arrays) and
  divided by the bag counts, matching the reference's fp32 semantics.

All data-dependent behavior (ragged boundaries) lives in per-core *data*
(segment ids), so a single SPMD program serves all cores.
"""
import numpy as np

from concourse import mybir
from concourse.bacc import Bacc
from concourse.tile import TileContext
from concourse.bass_utils import run_bass_kernel_spmd

# Hardcoded problem shape (harness contract).
B = 64          # bags
D = 512         # feature dim
T = 131072      # total rows
N_CORES = 8
P = 128         # SBUF partitions
T_LOCAL = T // N_CORES      # 16384 rows per core
NTILES = T_LOCAL // P       # 128 tiles of 128 rows
CHUNK = 8                   # tiles per DMA (8 * 256 KiB = 2 MiB)
NCHUNKS = NTILES // CHUNK   # 16

_NC_CACHE = None


def build_program(loop_repeats: int = 1, mm_fp32r: bool = False):
    """loop_repeats > 1 wraps the body in a hardware loop that redoes the
    identical work (same inputs/outputs) — used only for slope-based timing.
    mm_fp32r: run matmuls with operands bitcast to float32r (PE single-pass
    fp32 path, 4x faster at N=512; precision must be validated on HW)."""
    nc = Bacc()
    x = nc.dram_tensor("x", [T_LOCAL, D], mybir.dt.float32, kind="ExternalInput")
    # consts[:, :NTILES] = per-tile segment ids (transposed), consts[:, NTILES:]
    # = bag-index iota. One tensor so downstream ops depend on a single DMA.
    consts = nc.dram_tensor(
        "consts", [P, NTILES + B], mybir.dt.float32, kind="ExternalInput"
    )
    out = nc.dram_tensor("out", [B, D], mybir.dt.float32, kind="ExternalOutput")

    # [128, NTILES, 512]: element [p, c, d] = x[c*128 + p, d]
    x_view = x.rearrange("(c p) d -> p c d", p=P)

    with TileContext(nc) as tc:
        with (
            tc.tile_pool(name="const", bufs=1) as const_pool,
            tc.tile_pool(name="xin", bufs=3) as x_pool,
            tc.tile_pool(name="ind", bufs=3) as ind_pool,
            tc.tile_pool(name="psum", bufs=1, space="PSUM") as psum_pool,
            tc.tile_pool(name="outp", bufs=1) as out_pool,
        ):
            const_tile = const_pool.tile([P, NTILES + B], mybir.dt.float32)
            nc.sync.dma_start(out=const_tile[:], in_=consts[:])
            seg_tile = const_tile[:, :NTILES]
            iota_tile = const_tile[:, NTILES:]

            def body():
                acc = psum_pool.tile([B, D], mybir.dt.float32)
                for ch in range(NCHUNKS):
                    xt = x_pool.tile([P, CHUNK * D], mybir.dt.float32)
                    nc.sync.dma_start(
                        out=xt[:].rearrange("p (c d) -> p c d", c=CHUNK),
                        in_=x_view[:, ch * CHUNK : (ch + 1) * CHUNK, :],
                    )
                    ind = ind_pool.tile([P, CHUNK * B], mybir.dt.float32)
                    for c in range(CHUNK):
                        k = ch * CHUNK + c
                        nc.vector.tensor_tensor(
                            out=ind[:, c * B : (c + 1) * B],
                            in0=seg_tile[:, k : k + 1].to_broadcast([P, B]),
                            in1=iota_tile[:, :],
                            op=mybir.AluOpType.is_equal,
                        )
                    for c in range(CHUNK):
                        k = ch * CHUNK + c
                        lhsT = ind[:, c * B : (c + 1) * B]
                        rhs = xt[:, c * D : (c + 1) * D]
                        if mm_fp32r:
                            lhsT = lhsT.bitcast(mybir.dt.float32r)
                            rhs = rhs.bitcast(mybir.dt.float32r)
                        nc.tensor.matmul(
                            acc[:],
                            lhsT=lhsT,
                            rhs=rhs,
                            start=(k == 0),
                            stop=(k == NTILES - 1),
                        )

                out_sb = out_pool.tile([B, D], mybir.dt.float32)
                nc.vector.tensor_copy(out=out_sb[:], in_=acc[:])
                nc.sync.dma_start(out=out[:], in_=out_sb[:])

            if loop_repeats == 1:
                body()
            else:
                with tc.For_i(0, loop_repeats, 1):
                    body()

    nc.finalize()
    return nc


def _segment_ids(bags_num_samples: np.ndarray) -> np.ndarray:
    """Replicates jnp.repeat(arange(B), counts, total_repeat_length=T):
    truncate if the full repeat exceeds T, pad with the last value if short."""
    counts = np.asarray(bags_num_samples, dtype=np.int64)
    reps = np.repeat(np.arange(counts.shape[0], dtype=np.int64), np.maximum(counts, 0))
    if reps.size >= T:
        return reps[:T]
    pad_val = reps[-1] if reps.size else np.int64(0)
    return np.concatenate([reps, np.full(T - reps.size, pad_val, dtype=np.int64)])


def prepare_in_maps(samples: np.ndarray, bags_num_samples: np.ndarray):
    samples = np.ascontiguousarray(np.asarray(samples, dtype=np.float32))
    seg = _segment_ids(bags_num_samples).astype(np.float32)
    biota = np.broadcast_to(np.arange(B, dtype=np.float32)[None, :], (P, B))
    in_maps = []
    for i in range(N_CORES):
        lo = i * T_LOCAL
        seg_t = seg[lo : lo + T_LOCAL].reshape(NTILES, P).T  # [128, NTILES]
        consts = np.ascontiguousarray(np.concatenate([seg_t, biota], axis=1))
        in_maps.append({"x": samples[lo : lo + T_LOCAL], "consts": consts})
    return in_maps


def _combine(partials, bags_num_samples: np.ndarray) -> np.ndarray:
    sums = np.sum(np.stack(partials), axis=0, dtype=np.float32)
    counts_f = np.asarray(bags_num_samples)[:, None].astype(np.float32)
    with np.errstate(divide="ignore", invalid="ignore"):
        return (sums / counts_f).astype(np.float32)


def kernel(samples: np.ndarray, bags_num_samples: np.ndarray) -> np.ndarray:
    global _NC_CACHE
    if _NC_CACHE is None:
        _NC_CACHE = build_program()
    in_maps = prepare_in_maps(samples, bags_num_samples)
    res = run_bass_kernel_spmd(_NC_CACHE, in_maps, list(range(N_CORES)))
    partials = [res.results[i]["out"] for i in range(N_CORES)]
    return _combine(partials, bags_num_samples)
